# revision 38
# baseline (speedup 1.0000x reference)
"""Grouped MoE (top-2 of 8 experts, SwiGLU) on 8 Trainium2 NeuronCores.

Expert-parallel with host routing (gate on host, exact). Core c owns
expert c; tokens are gathered per expert into a fixed-capacity [D, cap]
buffer with capacity factor 1.0 (cap = mean load T*K/E): overflow pairs
of over-subscribed experts (~1%) are computed on host in exact fp32, the
standard MoE capacity/overflow design, so no core pads to the max expert
count. On device each core runs the three SwiGLU GEMMs in bf16 and
writes an UNSCALED output in a packed D-transposed layout; the host
de-interleaves, applies the per-token gate weight and scatter-adds the
two expert contributions. No collectives.

Layout/schedule design (all trace-driven; see the HAM warning below):
 - All DRAM tensors are packed partition-major so every DMA moves
   0.9-16 KB contiguous rows (~128 descriptors/transfer, full HBM rate;
   the per-queue descriptor rate makes <512B rows the bottleneck
   otherwise). w1/w3 are packed in per-f-tile blocks so the first
   A-stage matmul is gated on ~0.9 MB instead of ~2 MB.
 - Y-stage is w2-stationary (output [D-tile, tokens]): no partial
   m-tiles, arbitrary chunk sizes, gate scale moves to the host.
 - Stores batch 4 D-tiles into one [128, 4*tcz] tile per (chunk, half)
   (a store costs 128 descriptors regardless of width); the final store
   issues from scalar, in-order after its own dt3 copy.
 - The 14 x 512-col PE warm-up is LOAD-BEARING: the HAM latches the max
   p-state only after ~5us of continuous PE activity ending with ZERO
   idle gap into the real stream. Shorter warm-ups or any handoff gap
   latch ~2.0 GHz instead of ~2.35 GHz for the WHOLE run (+17us).
   Warm-up overshooting the DMA-ready moment is protective, not waste.
 - All dma_starts stay on nc.sync: its sequencer issues at ~0.1us while
   other engines only come online at 5-8us.
"""

import sys
import numpy as np

for _p in ("/opt/trn_rl_repo",):
    if _p not in sys.path:
        sys.path.insert(0, _p)

B, S, D, F, E, K = 2, 2048, 1024, 1024, 8, 2
T = B * S            # 4096 tokens
NCORES = 8
P = 128
DK = D // P          # 8 contraction chunks over D
FK = F // P          # 8 F tiles
BLK = DK * P         # w1/w3 f-block stride (k-major within a block)
NWARM = 14           # PE warm-up matmuls while the first DMAs land.
                     # 12 covers DMA-ready in a healthy device state, but in
                     # a slow DVFS state the DMA runs at ~half rate and 12
                     # ends ~0.8us early -> idle gap -> the HAM latches
                     # ~1.95GHz for the WHOLE run (+17-20us). Two extra
                     # warmups cost ~0.4us on good runs and close that gap.

_cache = {}


def _chunks(cap):
    """Token chunks <= 512 (PSUM bank limit), first/last kept small-ish.

    A/B and Y matmul cost is proportional to total tokens for any chunk
    >= ~192 (LDWEIGHTS hides under the column stream), so only the first
    chunk (gates the DMA lead-in) and last chunk (gates the tail) matter.
    """
    if cap <= 512:
        sizes = [cap]
    elif cap <= 832:
        sizes = [(cap + 1) // 2, cap // 2]
    else:
        # first chunk small: the early window is DMA-delivery-bound at
        # ~230GB/s, so x-chunk0's bytes directly delay the whole schedule
        sizes = [256]
        rem = cap - 256
        while rem - 512 >= 224:      # 224: LDWEIGHTS hides under >=~228 cols
            sizes.append(512)
            rem -= 512
        if rem > 512:
            sizes += [rem - 256, 256]
        else:
            sizes.append(rem)
    out = []
    o = 0
    for s in sizes:
        out.append((o, s))
        o += s
    assert o == cap and all(0 < s <= 512 for _, s in out)
    return out


def _build_nc(cap):
    from contextlib import ExitStack

    import concourse.mybir as mybir
    import concourse.tile as tile
    from concourse import bacc

    dt = mybir.dt
    AF = mybir.ActivationFunctionType
    ALU = mybir.AluOpType

    chunks = _chunks(cap)

    nc = bacc.Bacc("TRN2", target_bir_lowering=False, debug=False,
                   num_devices=NCORES)

    # all partition-major: row p holds that partition's full data span
    xh = nc.dram_tensor("xh", [P, DK * cap], dt.bfloat16,
                        kind="ExternalInput").ap()
    w1h = nc.dram_tensor("w1h", [P, FK * BLK], dt.bfloat16,
                         kind="ExternalInput").ap()
    w3h = nc.dram_tensor("w3h", [P, FK * BLK], dt.bfloat16,
                         kind="ExternalInput").ap()
    w2h = nc.dram_tensor("w2h", [P, FK * D], dt.bfloat16,
                         kind="ExternalInput").ap()
    # out is packed [(chunk)(dhalf)(dtile)(tok)] so every store DMA moves
    # one [128, 4*tcz] tile with 1.8-4KB contiguous rows (the per-dtile
    # layout would fragment rows to 456B and go descriptor-rate-bound);
    # the host de-interleaves during combine.
    out = nc.dram_tensor("out", [P, DK * cap], dt.bfloat16,
                         kind="ExternalOutput").ap()

    with tile.TileContext(nc) as tc, ExitStack() as ctx:
        # two pools total (per-tag bufs overrides) — each pool costs
        # alloc/release sync chains across every engine at kernel start/end
        sb = ctx.enter_context(tc.tile_pool(name="sb", bufs=1))
        ps = ctx.enter_context(tc.tile_pool(name="ps", bufs=2, space="PSUM"))

        x_sb = sb.tile([P, DK * cap], dt.bfloat16, tag="xall")
        w1_sb = sb.tile([P, FK * BLK], dt.bfloat16, tag="w1")
        w3_sb = sb.tile([P, FK * BLK], dt.bfloat16, tag="w3")
        w2_sb = sb.tile([P, FK * D], dt.bfloat16, tag="w2")

        # ---- DMA issue order = criticality order. Every transfer below is
        # 128 descriptors of >= 2KB contiguous rows (full HBM rate).
        # First A-group (f=0) is gated only on x-chunk0 + w1/w3 f0 blocks
        # (~1 MB); remaining f-blocks stream in ahead of the compute. ----
        # All DMA on the sync queue: its sequencer starts issuing at ~0.1us
        # while the other engines only come online at 5-8us, so anything
        # issued elsewhere would land BEHIND these transfers in the rings.
        # w1f0, w1f1 ahead of w3f0: the early-window DMA runs at ~230GB/s,
        # so each 1.08us A-group covers exactly one 0.25MB block in flight
        # (the first chunk computes psA-f0, psA-f1, psB-f0, psB-f1 in that
        # order to match; a psB-f0-right-after-psA-f0 order stalls ~0.5us)
        o0, tc0 = chunks[0]
        nc.sync.dma_start(x_sb[:, 0:DK * tc0], xh[:, 0:DK * tc0])
        nc.sync.dma_start(w1_sb[:, 0:BLK], w1h[:, 0:BLK])
        nc.sync.dma_start(w1_sb[:, BLK:2 * BLK], w1h[:, BLK:2 * BLK])
        nc.sync.dma_start(w3_sb[:, 0:BLK], w3h[:, 0:BLK])
        nc.sync.dma_start(w3_sb[:, BLK:2 * BLK], w3h[:, BLK:2 * BLK])
        for f in range(2, FK):
            nc.sync.dma_start(w1_sb[:, f * BLK:(f + 1) * BLK],
                              w1h[:, f * BLK:(f + 1) * BLK])
            nc.sync.dma_start(w3_sb[:, f * BLK:(f + 1) * BLK],
                              w3h[:, f * BLK:(f + 1) * BLK])
        # w2 in fk-halves; the Y loop consumes fk 0..3 before 4..7
        nc.sync.dma_start(w2_sb[:, 0:4 * D], w2h[:, 0:4 * D])
        nc.sync.dma_start(w2_sb[:, 4 * D:8 * D], w2h[:, 4 * D:8 * D])
        for (o, tcz) in chunks[1:]:
            nc.sync.dma_start(x_sb[:, DK * o:DK * (o + tcz)],
                              xh[:, DK * o:DK * (o + tcz)])

        # ---- PE warm-up: dummy matmuls while the first DMAs land keep the
        # HAM activity window full so the PE reaches max p-state ----
        wrm = sb.tile([P, 512], dt.bfloat16, tag="wrm")
        nc.vector.memset(wrm[:], 0.5)
        for _ in range(NWARM):
            psW = ps.tile([P, 512], dt.float32, tag="psA", name="psW")
            nc.tensor.matmul(psW[:], lhsT=wrm[:, 0:P], rhs=wrm[:],
                             start=True, stop=True)

        # ---- per-chunk SwiGLU FFN ----
        for ci, (o, tcz) in enumerate(chunks):
            xo = DK * o

            def grp(wsb, f, tag):
                pst = ps.tile([P, tcz], dt.float32, tag=tag, name=tag)
                for k in range(DK):
                    nc.tensor.matmul(
                        pst[:], lhsT=wsb[:, f * BLK + k * P:f * BLK + (k + 1) * P],
                        rhs=x_sb[:, xo + k * tcz:xo + (k + 1) * tcz],
                        start=(k == 0), stop=(k == DK - 1))
                return pst

            def act(f, psA, psB, h_sb):
                ssb = sb.tile([P, tcz], dt.bfloat16, tag="ssb", bufs=2)
                nc.scalar.activation(ssb[:], psA[:], AF.Silu)
                hsb = sb.tile([P, tcz], dt.bfloat16, tag=f"h{f}", bufs=2)
                nc.vector.tensor_tensor(hsb[:], ssb[:], psB[:], op=ALU.mult)
                h_sb.append(hsb)

            h_sb = []
            if ci == 0:
                # A0, A1, B0, B1: each group's weights are one DMA block
                # back, matching the early-window delivery rate (see the
                # DMA-order comment above)
                psA0 = grp(w1_sb, 0, "psA")
                psA1 = grp(w1_sb, 1, "psA")
                psB0 = grp(w3_sb, 0, "psB")
                act(0, psA0, psB0, h_sb)
                psB1 = grp(w3_sb, 1, "psB")
                act(1, psA1, psB1, h_sb)
                frest = range(2, FK)
            else:
                frest = range(FK)
            for f in frest:
                psA = grp(w1_sb, f, "psA")
                psB = grp(w3_sb, f, "psB")
                act(f, psA, psB, h_sb)
            # Y-stage, w2-stationary: psY[dt] = sum_fk w2T[fk, dtile] @ h[fk]
            # fkh-outer so the first half only needs w2 cols 0..4D
            for dhalf in range(2):
                psY = [ps.tile([P, tcz], dt.float32, tag="psY", bufs=4,
                                  name=f"psY{dhalf}_{i}") for i in range(4)]
                for fkh in range(2):
                    for dt_ in range(4):
                        dglob = dhalf * 4 + dt_
                        for fk in range(fkh * 4, fkh * 4 + 4):
                            nc.tensor.matmul(
                                psY[dt_][:],
                                lhsT=w2_sb[:, fk * D + dglob * P:fk * D + dglob * P + P],
                                rhs=h_sb[fk][:],
                                start=(fk == 0), stop=(fk == FK - 1))
                ysb = sb.tile([P, 4 * tcz], dt.bfloat16, tag="ysb", bufs=3)
                fin = o + tcz == cap and dhalf == 1
                for dt_ in range(4):
                    # alternate copy engines (gpsimd cannot access PSUM on
                    # TRN2, verified: birverifier rejects it). For the final
                    # group, dt3 goes on scalar and dt2 on vector so scalar
                    # is free the moment dt3's PSUM group stops.
                    dst = ysb[:, dt_ * tcz:(dt_ + 1) * tcz]
                    on_scalar = (dt_ % 2 == 0) if not fin else dt_ in (0, 3)
                    if on_scalar:
                        nc.scalar.activation(dst, psY[dt_][:], AF.Copy)
                    else:
                        nc.vector.tensor_scalar_mul(dst, psY[dt_][:], 1.0)
                # one store per (chunk, dhalf): every store costs 128
                # descriptors (one per partition) no matter its width, and
                # the tail drains at the per-queue descriptor rate — so
                # fewer, fatter stores beat early-issued split ones. The
                # final store issues from scalar: in-order after its own
                # dt3 copy, skipping the copy->DMA-queue semaphore hop.
                base = DK * o + dhalf * 4 * tcz
                eng = nc.scalar if fin else nc.sync
                eng.dma_start(out[:, base:base + 4 * tcz], ysb[:])

    nc.compile()
    return nc


def _route(xf, gate_w):
    """Host gate: returns per-expert (token indices, renormalized weights)."""
    logits = xf.astype(np.float64) @ gate_w.astype(np.float64).T   # [T, E]
    order = np.argsort(-logits, axis=1, kind="stable")
    i1 = order[:, 0]
    i2 = order[:, 1]
    ar = np.arange(T)
    l1 = logits[ar, i1]
    l2 = logits[ar, i2]
    g1 = 1.0 / (1.0 + np.exp(l2 - l1))
    g2 = 1.0 - g1
    idx_e, scl_e = [], []
    for e in range(E):
        m1 = i1 == e
        m2 = i2 == e
        ids = np.concatenate([np.nonzero(m1)[0], np.nonzero(m2)[0]])
        sc = np.concatenate([g1[m1], g2[m2]])
        idx_e.append(ids)
        scl_e.append(sc.astype(np.float32))
    return idx_e, scl_e


def prepare(x, gate_w, w1, w3, w2):
    """Host routing + sharding.

    Returns (nc, in_maps, (idx_e, scl_e, chunks, ovf)) where ovf holds
    the host-computed overflow contributions (capacity factor 1.0).
    """
    import ml_dtypes

    xf = np.ascontiguousarray(x.reshape(T, D).astype(np.float32))
    xTb = np.ascontiguousarray(xf.T).astype(ml_dtypes.bfloat16)   # [D, T]

    idx_e, scl_e = _route(xf, gate_w)
    maxcnt = max(len(i) for i in idx_e)
    # Capacity-factor-1.0 expert parallelism: device capacity is the MEAN
    # load T*K/E. Overflow pairs of over-subscribed experts (~1% here) are
    # computed on host in fp32 during combine (exact, so accuracy only
    # improves) instead of padding every core to the max expert count.
    cap = min(((maxcnt + 3) // 4) * 4, T * K // E)
    chunks = _chunks(cap)

    ovf = []
    for e in range(E):
        if len(idx_e[e]) > cap:
            sel = idx_e[e][cap:]
            sc = scl_e[e][cap:]
            xs = xf[sel].astype(np.float32)                      # [n, D]
            a = xs @ w1[e].T.astype(np.float32)
            b = xs @ w3[e].T.astype(np.float32)
            h = (a / (1.0 + np.exp(-a))) * b
            y = h @ w2[e].T.astype(np.float32)                   # [n, D]
            ovf.append((sel, sc[:, None] * y))
            idx_e[e] = idx_e[e][:cap]
            scl_e[e] = scl_e[e][:cap]

    if cap not in _cache:
        _cache[cap] = _build_nc(cap)
    nc = _cache[cap]

    in_maps = []
    for c in range(NCORES):
        ids = idx_e[c]
        cnt = len(ids)
        xg = np.zeros((D, cap), dtype=ml_dtypes.bfloat16)
        xg[:, :cnt] = xTb[:, ids]
        # chunk-major, then k-major partition blocks: chunk rows contiguous
        xh = np.concatenate([
            xg[:, o:o + tcz].reshape(DK, P, tcz).transpose(1, 0, 2)
            .reshape(P, DK * tcz) for (o, tcz) in chunks], axis=1)

        w1T = np.ascontiguousarray(w1[c].T).astype(ml_dtypes.bfloat16)  # [D,F]
        w3T = np.ascontiguousarray(w3[c].T).astype(ml_dtypes.bfloat16)
        w2T = np.ascontiguousarray(w2[c].T).astype(ml_dtypes.bfloat16)  # [F,D]

        def fmaj(wT):
            # [D, F] -> [128, f-major [f][k][128]] per-f-tile blocks
            return np.concatenate([
                wT[:, f * P:(f + 1) * P].reshape(DK, P, P).transpose(1, 0, 2)
                .reshape(P, BLK) for f in range(FK)], axis=1)

        in_maps.append({
            "xh": np.ascontiguousarray(xh),
            "w1h": fmaj(w1T),
            "w3h": fmaj(w3T),
            "w2h": np.ascontiguousarray(
                w2T.reshape(FK, P, D).transpose(1, 0, 2).reshape(P, FK * D)),
        })
    return nc, in_maps, (idx_e, scl_e, chunks, ovf)


def _combine(res, meta):
    idx_e, scl_e, chunks, ovf = meta
    outf = np.zeros((T, D), dtype=np.float32)
    for sel, contrib in ovf:
        outf[sel] += contrib
    for c in range(NCORES):
        cnt = len(idx_e[c])
        raw = res.results[c]["out"].astype(np.float32)   # [128, 8*cap]
        cap = raw.shape[1] // DK
        y = np.empty((D, cap), dtype=np.float32)
        for (o, tcz) in chunks:
            blk = raw[:, DK * o:DK * (o + tcz)].reshape(P, 8, tcz)
            for dglob in range(8):
                y[dglob * P:(dglob + 1) * P, o:o + tcz] = blk[:, dglob, :]
        outf[idx_e[c]] += scl_e[c][:, None] * y[:, :cnt].T
    return outf.reshape(B, S, D)


def kernel(x, gate_w, w1, w3, w2):
    from concourse.bass_utils import run_bass_kernel_spmd

    nc, in_maps, meta = prepare(x, gate_w, w1, w3, w2)
    res = run_bass_kernel_spmd(nc, in_maps, list(range(NCORES)))
    return _combine(res, meta)


# revision 39
# speedup vs baseline: 1.0195x; 1.0195x over previous
"""Grouped MoE (top-2 of 8 experts, SwiGLU) on 8 Trainium2 NeuronCores.

Expert-parallel with host routing (gate on host, exact). Core c owns
expert c; tokens are gathered per expert into a fixed-capacity [D, cap]
buffer with capacity factor 1.0 (cap = mean load T*K/E): overflow pairs
of over-subscribed experts (~1%) are computed on host in exact fp32, the
standard MoE capacity/overflow design, so no core pads to the max expert
count. On device each core runs the three SwiGLU GEMMs in bf16 and
writes an UNSCALED output in a packed D-transposed layout; the host
de-interleaves, applies the per-token gate weight and scatter-adds the
two expert contributions. No collectives.

Layout/schedule design (all trace-driven; see the HAM warning below):
 - All DRAM tensors are packed partition-major so every DMA moves
   0.9-16 KB contiguous rows (~128 descriptors/transfer, full HBM rate;
   the per-queue descriptor rate makes <512B rows the bottleneck
   otherwise). w1/w3 are packed in per-f-tile blocks so the first
   A-stage matmul is gated on ~0.9 MB instead of ~2 MB.
 - Y-stage is w2-stationary (output [D-tile, tokens]): no partial
   m-tiles, arbitrary chunk sizes, gate scale moves to the host.
 - Stores batch 4 D-tiles into one [128, 4*tcz] tile per (chunk, half)
   (a store costs 128 descriptors regardless of width); the final store
   issues from scalar, in-order after its own dt3 copy.
 - The 14 x 512-col PE warm-up is LOAD-BEARING: the HAM latches the max
   p-state only after ~5us of continuous PE activity ending with ZERO
   idle gap into the real stream. Shorter warm-ups or any handoff gap
   latch ~2.0 GHz instead of ~2.35 GHz for the WHOLE run (+17us).
   Warm-up overshooting the DMA-ready moment is protective, not waste.
 - All dma_starts stay on nc.sync: its sequencer issues at ~0.1us while
   other engines only come online at 5-8us.
"""

import sys
import numpy as np

for _p in ("/opt/trn_rl_repo",):
    if _p not in sys.path:
        sys.path.insert(0, _p)

B, S, D, F, E, K = 2, 2048, 1024, 1024, 8, 2
T = B * S            # 4096 tokens
NCORES = 8
P = 128
DK = D // P          # 8 contraction chunks over D
FK = F // P          # 8 F tiles
BLK = DK * P         # w1/w3 f-block stride (k-major within a block)
NWARM = 14           # PE warm-up matmuls while the first DMAs land.
                     # 12 covers DMA-ready in a healthy device state, but in
                     # a slow DVFS state the DMA runs at ~half rate and 12
                     # ends ~0.8us early -> idle gap -> the HAM latches
                     # ~1.95GHz for the WHOLE run (+17-20us). Two extra
                     # warmups cost ~0.4us on good runs and close that gap.

_cache = {}


def _chunks(cap):
    """Token chunks <= 512 (PSUM bank limit), first/last kept small-ish.

    A/B and Y matmul cost is proportional to total tokens for any chunk
    >= ~192 (LDWEIGHTS hides under the column stream), so only the first
    chunk (gates the DMA lead-in) and last chunk (gates the tail) matter.
    """
    if cap <= 512:
        sizes = [cap]
    elif cap <= 832:
        sizes = [(cap + 1) // 2, cap // 2]
    else:
        # first chunk 320: large enough that chunk0's compute window covers
        # the mid-window loads (w2 + x-chunk1; a 256 first chunk starves
        # Y-ch0/A-B-ch1), small enough to keep the early DMA lead-in short
        sizes = [320]
        rem = cap - 320
        while rem - 512 >= 224:      # 224: LDWEIGHTS hides under >=~228 cols
            sizes.append(512)
            rem -= 512
        if rem > 512:
            sizes += [rem - 256, 256]
        else:
            sizes.append(rem)
    out = []
    o = 0
    for s in sizes:
        out.append((o, s))
        o += s
    assert o == cap and all(0 < s <= 512 for _, s in out)
    return out


def _build_nc(cap):
    from contextlib import ExitStack

    import concourse.mybir as mybir
    import concourse.tile as tile
    from concourse import bacc

    dt = mybir.dt
    AF = mybir.ActivationFunctionType
    ALU = mybir.AluOpType

    chunks = _chunks(cap)

    nc = bacc.Bacc("TRN2", target_bir_lowering=False, debug=False,
                   num_devices=NCORES)

    # all partition-major: row p holds that partition's full data span
    xh = nc.dram_tensor("xh", [P, DK * cap], dt.bfloat16,
                        kind="ExternalInput").ap()
    w1h = nc.dram_tensor("w1h", [P, FK * BLK], dt.bfloat16,
                         kind="ExternalInput").ap()
    w3h = nc.dram_tensor("w3h", [P, FK * BLK], dt.bfloat16,
                         kind="ExternalInput").ap()
    w2h = nc.dram_tensor("w2h", [P, FK * D], dt.bfloat16,
                         kind="ExternalInput").ap()
    # out is packed [(chunk)(dhalf)(dtile)(tok)] so every store DMA moves
    # one [128, 4*tcz] tile with 1.8-4KB contiguous rows (the per-dtile
    # layout would fragment rows to 456B and go descriptor-rate-bound);
    # the host de-interleaves during combine.
    out = nc.dram_tensor("out", [P, DK * cap], dt.bfloat16,
                         kind="ExternalOutput").ap()

    with tile.TileContext(nc) as tc, ExitStack() as ctx:
        # two pools total (per-tag bufs overrides) — each pool costs
        # alloc/release sync chains across every engine at kernel start/end
        sb = ctx.enter_context(tc.tile_pool(name="sb", bufs=1))
        ps = ctx.enter_context(tc.tile_pool(name="ps", bufs=2, space="PSUM"))

        x_sb = sb.tile([P, DK * cap], dt.bfloat16, tag="xall")
        w1_sb = sb.tile([P, FK * BLK], dt.bfloat16, tag="w1")
        w3_sb = sb.tile([P, FK * BLK], dt.bfloat16, tag="w3")
        w2_sb = sb.tile([P, FK * D], dt.bfloat16, tag="w2")

        # ---- DMA issue order = criticality order. Every transfer below is
        # 128 descriptors of >= 2KB contiguous rows (full HBM rate).
        # First A-group (f=0) is gated only on x-chunk0 + w1/w3 f0 blocks
        # (~1 MB); remaining f-blocks stream in ahead of the compute. ----
        # All DMA on the sync queue: its sequencer starts issuing at ~0.1us
        # while the other engines only come online at 5-8us, so anything
        # issued elsewhere would land BEHIND these transfers in the rings.
        # w1f0, w1f1 ahead of w3f0: the early-window DMA runs at ~230GB/s,
        # so each 1.08us A-group covers exactly one 0.25MB block in flight
        # (the first chunk computes psA-f0, psA-f1, psB-f0, psB-f1 in that
        # order to match; a psB-f0-right-after-psA-f0 order stalls ~0.5us)
        o0, tc0 = chunks[0]
        nc.sync.dma_start(x_sb[:, 0:DK * tc0], xh[:, 0:DK * tc0])
        nc.sync.dma_start(w1_sb[:, 0:BLK], w1h[:, 0:BLK])
        nc.sync.dma_start(w1_sb[:, BLK:2 * BLK], w1h[:, BLK:2 * BLK])
        nc.sync.dma_start(w3_sb[:, 0:BLK], w3h[:, 0:BLK])
        nc.sync.dma_start(w3_sb[:, BLK:2 * BLK], w3h[:, BLK:2 * BLK])
        for f in range(2, FK):
            nc.sync.dma_start(w1_sb[:, f * BLK:(f + 1) * BLK],
                              w1h[:, f * BLK:(f + 1) * BLK])
            nc.sync.dma_start(w3_sb[:, f * BLK:(f + 1) * BLK],
                              w3h[:, f * BLK:(f + 1) * BLK])
        # w2 in fk-halves; the Y loop consumes fk 0..3 before 4..7
        nc.sync.dma_start(w2_sb[:, 0:4 * D], w2h[:, 0:4 * D])
        nc.sync.dma_start(w2_sb[:, 4 * D:8 * D], w2h[:, 4 * D:8 * D])
        for (o, tcz) in chunks[1:]:
            nc.sync.dma_start(x_sb[:, DK * o:DK * (o + tcz)],
                              xh[:, DK * o:DK * (o + tcz)])

        # ---- PE warm-up: dummy matmuls while the first DMAs land keep the
        # HAM activity window full so the PE reaches max p-state ----
        wrm = sb.tile([P, 512], dt.bfloat16, tag="wrm")
        nc.vector.memset(wrm[:], 0.5)
        for _ in range(NWARM):
            psW = ps.tile([P, 512], dt.float32, tag="psA", name="psW")
            nc.tensor.matmul(psW[:], lhsT=wrm[:, 0:P], rhs=wrm[:],
                             start=True, stop=True)

        # ---- per-chunk SwiGLU FFN ----
        for ci, (o, tcz) in enumerate(chunks):
            xo = DK * o

            def grp(wsb, f, tag):
                pst = ps.tile([P, tcz], dt.float32, tag=tag, name=tag)
                for k in range(DK):
                    nc.tensor.matmul(
                        pst[:], lhsT=wsb[:, f * BLK + k * P:f * BLK + (k + 1) * P],
                        rhs=x_sb[:, xo + k * tcz:xo + (k + 1) * tcz],
                        start=(k == 0), stop=(k == DK - 1))
                return pst

            def act(f, psA, psB, h_sb):
                ssb = sb.tile([P, tcz], dt.bfloat16, tag="ssb", bufs=2)
                nc.scalar.activation(ssb[:], psA[:], AF.Silu)
                hsb = sb.tile([P, tcz], dt.bfloat16, tag=f"h{f}", bufs=2)
                nc.vector.tensor_tensor(hsb[:], ssb[:], psB[:], op=ALU.mult)
                h_sb.append(hsb)

            h_sb = []
            if ci == 0:
                # A0, A1, B0, B1: each group's weights are one DMA block
                # back, matching the early-window delivery rate (see the
                # DMA-order comment above)
                psA0 = grp(w1_sb, 0, "psA")
                psA1 = grp(w1_sb, 1, "psA")
                psB0 = grp(w3_sb, 0, "psB")
                act(0, psA0, psB0, h_sb)
                psB1 = grp(w3_sb, 1, "psB")
                act(1, psA1, psB1, h_sb)
                frest = range(2, FK)
            else:
                frest = range(FK)
            for f in frest:
                psA = grp(w1_sb, f, "psA")
                psB = grp(w3_sb, f, "psB")
                act(f, psA, psB, h_sb)
            # Y-stage, w2-stationary: psY[dt] = sum_fk w2T[fk, dtile] @ h[fk]
            # fkh-outer so the first half only needs w2 cols 0..4D
            for dhalf in range(2):
                psY = [ps.tile([P, tcz], dt.float32, tag="psY", bufs=4,
                                  name=f"psY{dhalf}_{i}") for i in range(4)]
                for fkh in range(2):
                    for dt_ in range(4):
                        dglob = dhalf * 4 + dt_
                        for fk in range(fkh * 4, fkh * 4 + 4):
                            nc.tensor.matmul(
                                psY[dt_][:],
                                lhsT=w2_sb[:, fk * D + dglob * P:fk * D + dglob * P + P],
                                rhs=h_sb[fk][:],
                                start=(fk == 0), stop=(fk == FK - 1))
                ysb = sb.tile([P, 4 * tcz], dt.bfloat16, tag="ysb", bufs=3)
                fin = o + tcz == cap and dhalf == 1
                for dt_ in range(4):
                    # alternate copy engines (gpsimd cannot access PSUM on
                    # TRN2, verified: birverifier rejects it). For the final
                    # group, dt3 goes on scalar and dt2 on vector so scalar
                    # is free the moment dt3's PSUM group stops.
                    dst = ysb[:, dt_ * tcz:(dt_ + 1) * tcz]
                    on_scalar = (dt_ % 2 == 0) if not fin else dt_ in (0, 3)
                    if on_scalar:
                        nc.scalar.activation(dst, psY[dt_][:], AF.Copy)
                    else:
                        nc.vector.tensor_scalar_mul(dst, psY[dt_][:], 1.0)
                # one store per (chunk, dhalf): every store costs 128
                # descriptors (one per partition) no matter its width, and
                # the tail drains at the per-queue descriptor rate — so
                # fewer, fatter stores beat early-issued split ones. The
                # final store issues from scalar: in-order after its own
                # dt3 copy, skipping the copy->DMA-queue semaphore hop.
                base = DK * o + dhalf * 4 * tcz
                eng = nc.scalar if fin else nc.sync
                eng.dma_start(out[:, base:base + 4 * tcz], ysb[:])

    nc.compile()
    return nc


def _route(xf, gate_w):
    """Host gate: returns per-expert (token indices, renormalized weights)."""
    logits = xf.astype(np.float64) @ gate_w.astype(np.float64).T   # [T, E]
    order = np.argsort(-logits, axis=1, kind="stable")
    i1 = order[:, 0]
    i2 = order[:, 1]
    ar = np.arange(T)
    l1 = logits[ar, i1]
    l2 = logits[ar, i2]
    g1 = 1.0 / (1.0 + np.exp(l2 - l1))
    g2 = 1.0 - g1
    idx_e, scl_e = [], []
    for e in range(E):
        m1 = i1 == e
        m2 = i2 == e
        ids = np.concatenate([np.nonzero(m1)[0], np.nonzero(m2)[0]])
        sc = np.concatenate([g1[m1], g2[m2]])
        idx_e.append(ids)
        scl_e.append(sc.astype(np.float32))
    return idx_e, scl_e


def prepare(x, gate_w, w1, w3, w2):
    """Host routing + sharding.

    Returns (nc, in_maps, (idx_e, scl_e, chunks, ovf)) where ovf holds
    the host-computed overflow contributions (capacity factor 1.0).
    """
    import ml_dtypes

    xf = np.ascontiguousarray(x.reshape(T, D).astype(np.float32))
    xTb = np.ascontiguousarray(xf.T).astype(ml_dtypes.bfloat16)   # [D, T]

    idx_e, scl_e = _route(xf, gate_w)
    maxcnt = max(len(i) for i in idx_e)
    # Capacity-factor-1.0 expert parallelism: device capacity is the MEAN
    # load T*K/E. Overflow pairs of over-subscribed experts (~1% here) are
    # computed on host in fp32 during combine (exact, so accuracy only
    # improves) instead of padding every core to the max expert count.
    cap = min(((maxcnt + 3) // 4) * 4, T * K // E)
    chunks = _chunks(cap)

    ovf = []
    for e in range(E):
        if len(idx_e[e]) > cap:
            sel = idx_e[e][cap:]
            sc = scl_e[e][cap:]
            xs = xf[sel].astype(np.float32)                      # [n, D]
            a = xs @ w1[e].T.astype(np.float32)
            b = xs @ w3[e].T.astype(np.float32)
            h = (a / (1.0 + np.exp(-a))) * b
            y = h @ w2[e].T.astype(np.float32)                   # [n, D]
            ovf.append((sel, sc[:, None] * y))
            idx_e[e] = idx_e[e][:cap]
            scl_e[e] = scl_e[e][:cap]

    if cap not in _cache:
        _cache[cap] = _build_nc(cap)
    nc = _cache[cap]

    in_maps = []
    for c in range(NCORES):
        ids = idx_e[c]
        cnt = len(ids)
        xg = np.zeros((D, cap), dtype=ml_dtypes.bfloat16)
        xg[:, :cnt] = xTb[:, ids]
        # chunk-major, then k-major partition blocks: chunk rows contiguous
        xh = np.concatenate([
            xg[:, o:o + tcz].reshape(DK, P, tcz).transpose(1, 0, 2)
            .reshape(P, DK * tcz) for (o, tcz) in chunks], axis=1)

        w1T = np.ascontiguousarray(w1[c].T).astype(ml_dtypes.bfloat16)  # [D,F]
        w3T = np.ascontiguousarray(w3[c].T).astype(ml_dtypes.bfloat16)
        w2T = np.ascontiguousarray(w2[c].T).astype(ml_dtypes.bfloat16)  # [F,D]

        def fmaj(wT):
            # [D, F] -> [128, f-major [f][k][128]] per-f-tile blocks
            return np.concatenate([
                wT[:, f * P:(f + 1) * P].reshape(DK, P, P).transpose(1, 0, 2)
                .reshape(P, BLK) for f in range(FK)], axis=1)

        in_maps.append({
            "xh": np.ascontiguousarray(xh),
            "w1h": fmaj(w1T),
            "w3h": fmaj(w3T),
            "w2h": np.ascontiguousarray(
                w2T.reshape(FK, P, D).transpose(1, 0, 2).reshape(P, FK * D)),
        })
    return nc, in_maps, (idx_e, scl_e, chunks, ovf)


def _combine(res, meta):
    idx_e, scl_e, chunks, ovf = meta
    outf = np.zeros((T, D), dtype=np.float32)
    for sel, contrib in ovf:
        outf[sel] += contrib
    for c in range(NCORES):
        cnt = len(idx_e[c])
        raw = res.results[c]["out"].astype(np.float32)   # [128, 8*cap]
        cap = raw.shape[1] // DK
        y = np.empty((D, cap), dtype=np.float32)
        for (o, tcz) in chunks:
            blk = raw[:, DK * o:DK * (o + tcz)].reshape(P, 8, tcz)
            for dglob in range(8):
                y[dglob * P:(dglob + 1) * P, o:o + tcz] = blk[:, dglob, :]
        outf[idx_e[c]] += scl_e[c][:, None] * y[:, :cnt].T
    return outf.reshape(B, S, D)


def kernel(x, gate_w, w1, w3, w2):
    from concourse.bass_utils import run_bass_kernel_spmd

    nc, in_maps, meta = prepare(x, gate_w, w1, w3, w2)
    res = run_bass_kernel_spmd(nc, in_maps, list(range(NCORES)))
    return _combine(res, meta)
